# revision 14
# baseline (speedup 1.0000x reference)
"""MoE-LoRA linear layer (T=16384, D=1024, E=64, R=8) on 8 Trainium2 cores.

Strategy: data-parallel over tokens (2048 tokens/core). Inside each core
everything is computed transposed (d on partitions, tokens on the free dim)
so every matmul consumes operands in their natural layout with no on-device
transposes:

  out_T[:, g] = sum_k W_k^T @ xT_k[:, g]      base GEMM, N=512 token groups
  out_T[:, b] += B_blk^T @ (mask_b * (A_blk^T @ xT[:, b]))   rank-8 LoRA

Routing is resolved on the host: each core's tokens are sorted by expert
label and cut into 256-token blocks; per block the (<=16) experts present
are packed into per-block A / B / mask tensors. The device program is thus
identical for all 8 cores (one SPMD NEFF) and all data-dependence lives in
input data. The LoRA matmul accumulates directly into the base GEMM's PSUM
tile (column sub-range), so composition costs no extra DVE work.

Schedule: the first token group's x/A/W stream as four combined k-pair
"waves" (one ~1.25MB DMA each) whose arrival rate matches PE consumption;
later groups' x and the B/mask tables arrive as single DMAs ordered by
first use. Throwaway warm-up matmuls bridge the fixed ~7.5us framework
preamble so the PE clock gate (HAM, 1.2 -> 2.4 GHz) releases early and
never re-throttles. Compute in bf16 (f32 PSUM): fp32 matmul on TRN2 runs
at 1/4 rate and would be hopelessly PE-bound.
"""

import numpy as np
import ml_dtypes

import concourse.bacc as bacc
import concourse.mybir as mybir
from concourse import tile
from concourse.bass_utils import run_bass_kernel_spmd

T, D, E, R = 16384, 1024, 64, 8
N_CORES = 8
TPC = T // N_CORES          # tokens per core
KD = D // 128               # 8 contraction chunks
KQ = KD // 2                # k-pair waves for the first group
GRP = 512                   # base-GEMM token group (one PSUM bank)
NG = TPC // GRP             # 4 groups
SCALING = 1.0 / R
SLOTS = 128 // R            # experts per lora block the packed layout holds

BF16 = ml_dtypes.bfloat16

_compiled = {}              # n_blocks -> Bacc program (reused across calls)
_last_in_maps = None


def _build_nc(n_blocks: int):
    blk = TPC // n_blocks   # lora block (256 default)
    sub = GRP // blk        # lora blocks per token group
    WV = 2 * GRP + n_blocks * 2 * 128 + 2 * D   # combined wave row: x | A | W
    LB = D + blk                                 # lora-table row per block: B | M
    bf = mybir.dt.bfloat16
    f32 = mybir.dt.float32

    nc = bacc.Bacc(
        "TRN2", target_bir_lowering=False, debug=False, num_devices=N_CORES
    )
    # host-packed SBUF layouts; every DMA source is contiguous per partition
    wv_d = nc.dram_tensor("wv", [KQ, 128, WV], bf, kind="ExternalInput")
    xr_d = nc.dram_tensor("xr", [NG - 1, 128, KD, GRP], bf, kind="ExternalInput")
    # lora tables in three pieces by first use: group0, group1, groups 2-3
    lt_shapes = [sub, sub, n_blocks - 2 * sub]
    lt_d = [
        nc.dram_tensor(f"lt{i}", [128, n * LB], bf, kind="ExternalInput")
        for i, n in enumerate(lt_shapes)
    ]
    bias_d = nc.dram_tensor("bias", [128, KD], f32, kind="ExternalInput")
    out_d = nc.dram_tensor("outT", [KD, 128, TPC], f32, kind="ExternalOutput")

    with tile.TileContext(nc) as tc:
        with (
            tc.tile_pool(name="consts", bufs=1) as cpool,
            tc.tile_pool(name="xa_ps", bufs=3, space="PSUM") as xa_ps,
            tc.tile_pool(name="out_ps", bufs=5, space="PSUM") as out_ps,
            tc.tile_pool(name="stage", bufs=4) as stage_pool,
        ):
            bias_t = cpool.tile([128, KD], f32, tag="bias", name="bias_t")
            wv_t = [
                cpool.tile([128, WV], bf, tag=f"wv{q}", name=f"wv_t{q}")
                for q in range(KQ)
            ]
            xr_t = [
                cpool.tile([128, KD * GRP], bf, tag=f"xr{g}", name=f"xr_t{g}")
                for g in range(1, NG)
            ]
            lt_t = [
                cpool.tile([128, n * LB], bf, tag=f"lt{i}", name=f"lt_t{i}")
                for i, n in enumerate(lt_shapes)
            ]
            warm_sb = cpool.tile([128, GRP], bf, tag="warm", name="warm_sb")

            A_OFF = 2 * GRP
            W_OFF = 2 * GRP + n_blocks * 2 * 128

            def a_sl(b, k):
                q, kk = divmod(k, 2)
                o = A_OFF + (b * 2 + kk) * 128
                return wv_t[q][:, o : o + 128]

            def w_sl(k, j):
                q, kk = divmod(k, 2)
                o = W_OFF + kk * D + j * 128
                return wv_t[q][:, o : o + 128]

            def x_sl(g, k, c0, c1):
                if g == 0:
                    q, kk = divmod(k, 2)
                    return wv_t[q][:, kk * GRP + c0 : kk * GRP + c1]
                return xr_t[g - 1][:, k * GRP + c0 : k * GRP + c1]

            def _lt(b):
                if b < sub:
                    return lt_t[0], b
                if b < 2 * sub:
                    return lt_t[1], b - sub
                return lt_t[2], b - 2 * sub

            def b_sl(b, j):
                t, lb = _lt(b)
                o = lb * LB + j * 128
                return t[:, o : o + 128]

            def m_sl(b):
                t, lb = _lt(b)
                o = lb * LB + D
                return t[:, o : o + blk]

            # issue order == arrival order (one sequencer queue)
            nc.sync.dma_start(bias_t[:], bias_d[:, :])
            for q in range(KQ):
                eng = nc.sync if q % 2 == 0 else nc.gpsimd
                eng.dma_start(wv_t[q][:], wv_d[q, :, :])
            nc.sync.dma_start(lt_t[0][:], lt_d[0][:, :])
            nc.sync.dma_start(xr_t[0][:], xr_d[0, :, :, :])
            nc.sync.dma_start(lt_t[1][:], lt_d[1][:, :])
            nc.sync.dma_start(xr_t[1][:], xr_d[1, :, :, :])
            nc.sync.dma_start(lt_t[2][:], lt_d[2][:, :])
            nc.sync.dma_start(xr_t[2][:], xr_d[2, :, :, :])

            # PE warm-up across the fixed framework preamble
            nc.vector.memset(warm_sb[:], 0.0)
            for _ in range(16):
                warm_ps = xa_ps.tile([128, GRP], f32, tag="xa", name="warm_ps")
                nc.tensor.matmul(
                    warm_ps[:],
                    lhsT=warm_sb[:, 0:128],
                    rhs=warm_sb[:],
                    start=True,
                    stop=True,
                    skip_group_check=True,
                )

            xa_m = [None] * n_blocks
            xa_p = [None] * n_blocks

            def emit_xa(b, ks, masked):
                # xa[slot, t] for lora block b over k-chunks ks
                g, h = divmod(b, sub)
                if ks[0] == 0:
                    xa_p[b] = xa_ps.tile([128, blk], f32, tag="xa", name=f"xa_p{b}")
                for k in ks:
                    nc.tensor.matmul(
                        xa_p[b][:],
                        lhsT=a_sl(b, k),
                        rhs=x_sl(g, k, h * blk, (h + 1) * blk),
                        start=(k == 0),
                        stop=(k == KD - 1),
                    )
                if masked:
                    xm = cpool.tile([128, blk], bf, tag=f"xam{b}", name=f"xm{b}")
                    nc.vector.tensor_mul(xm[:], xa_p[b][:], m_sl(b))
                    xa_m[b] = xm

            def emit_base(g, j, o_p, ks):
                for k in ks:
                    nc.tensor.matmul(
                        o_p[:],
                        lhsT=w_sl(k, j),
                        rhs=x_sl(g, k, 0, GRP),
                        start=(k == 0),
                        stop=False,
                        skip_group_check=True,
                    )

            def emit_lora_bias(g, j, o_p):
                for h in range(sub):
                    b = g * sub + h
                    nc.tensor.matmul(
                        o_p[:, h * blk : (h + 1) * blk],
                        lhsT=b_sl(b, j),
                        rhs=xa_m[b][:],
                        start=False,
                        stop=(h == sub - 1),
                        skip_group_check=True,
                    )
                st = stage_pool.tile([128, GRP], f32, tag="st", name=f"st{g}_{j}")
                nc.vector.tensor_scalar_add(st[:], o_p[:], bias_t[:, j : j + 1])
                nc.sync.dma_start(out_d[j, :, g * GRP : (g + 1) * GRP], st[:])

            # --- group 0: wave schedule matched to DMA arrivals ---
            o_p0 = {}
            for j in range(5):
                o_p0[j] = out_ps.tile([128, GRP], f32, tag="o", name=f"o_p0_{j}")
            for q in range(KQ):
                ks = (2 * q, 2 * q + 1)
                for j in range(5):
                    emit_base(0, j, o_p0[j], ks)
                for b in range(sub):
                    emit_xa(b, ks, masked=(q == KQ - 1))
            for j in range(5):
                emit_lora_bias(0, j, o_p0[j])
            for j in range(5, KD):
                o_p = out_ps.tile([128, GRP], f32, tag="o", name=f"o_p0_{j}")
                emit_base(0, j, o_p, range(KD))
                emit_lora_bias(0, j, o_p)
                if j == 5:
                    for h in range(sub):
                        emit_xa(sub + h, range(KD), masked=True)

            # --- groups 1..3: straight pipeline ---
            for g in range(1, NG):
                for j in range(KD):
                    o_p = out_ps.tile([128, GRP], f32, tag="o", name=f"o_p{g}_{j}")
                    emit_base(g, j, o_p, range(KD))
                    emit_lora_bias(g, j, o_p)
                    if j == 3 and g < NG - 1:
                        # next group's xa, placed where its x tile has arrived
                        for h in range(sub):
                            emit_xa((g + 1) * sub + h, range(KD), masked=True)

    nc.compile()
    return nc


def _pick_n_blocks(labels: np.ndarray) -> int:
    for n_blocks in (8, 16, 32, 64, 128, 256):
        blk = TPC // n_blocks
        ok = True
        for c in range(N_CORES):
            sl = np.sort(labels[c * TPC : (c + 1) * TPC])
            for b in range(n_blocks):
                if len(np.unique(sl[b * blk : (b + 1) * blk])) > SLOTS:
                    ok = False
                    break
            if not ok:
                break
        if ok:
            return n_blocks
    raise ValueError("could not find a block size with <=16 experts per block")


def kernel(x, labels, W, A, B, bias):
    global _last_in_maps
    x = np.asarray(x, dtype=np.float32)
    labels_i = np.asarray(labels).astype(np.int64)
    W = np.asarray(W, dtype=np.float32)
    A = np.asarray(A, dtype=np.float32)
    B = np.asarray(B, dtype=np.float32)
    bias = np.asarray(bias, dtype=np.float32)

    n_blocks = _pick_n_blocks(labels_i)
    blk = TPC // n_blocks
    sub = GRP // blk

    if n_blocks not in _compiled:
        _compiled[n_blocks] = _build_nc(n_blocks)
    nc = _compiled[n_blocks]

    # w_wave[q, p, kk, :] = W[128*(2q+kk)+p, :]
    w_wave = W.reshape(KQ, 2, 128, D).transpose(0, 2, 1, 3).astype(BF16)
    bias_in = np.ascontiguousarray(bias.reshape(KD, 128).T)  # [128, KD] f32
    B_scaled = (B * SCALING).astype(np.float32)

    in_maps = []
    perms = []
    for c in range(N_CORES):
        lc = labels_i[c * TPC : (c + 1) * TPC]
        perm = np.argsort(lc, kind="stable")
        perms.append(perm)
        ls = lc[perm]                          # sorted labels
        xs = x[c * TPC : (c + 1) * TPC][perm]  # [TPC, D] sorted tokens

        # xt_full[k, p, g, t] = xs[g*GRP + t, 128k + p]
        xt_full = xs.astype(BF16).T.reshape(KD, 128, NG, GRP)
        # x0 wave part [q, p, kk, t]
        x0_wave = xt_full[:, :, 0, :].reshape(KQ, 2, 128, GRP).transpose(0, 2, 1, 3)
        xr_in = np.ascontiguousarray(
            xt_full[:, :, 1:, :].transpose(2, 1, 0, 3)    # [NG-1, 128, KD, GRP]
        )

        # packed per-block expert tables
        a_pack = np.zeros((128, n_blocks, KD, 128), dtype=BF16)
        b_pack = np.zeros((128, n_blocks, D), dtype=BF16)
        m_pack = np.zeros((128, n_blocks, blk), dtype=BF16)
        for b in range(n_blocks):
            seg = ls[b * blk : (b + 1) * blk]
            experts = np.unique(seg)
            assert len(experts) <= SLOTS
            for i, e in enumerate(experts):
                # lhsT slot: a_pack[p, b, k, 8i+r] = A[e, 128k+p, r]
                a_pack[:, b, :, i * R : (i + 1) * R] = A[e].reshape(
                    KD, 128, R
                ).transpose(1, 0, 2)
                b_pack[i * R : (i + 1) * R, b, :] = B_scaled[e]
                m_pack[i * R : (i + 1) * R, b, :] = (seg == e)[None, :]

        # wave[q] = x0 | A | W  (concat along free dim)
        a_wave = a_pack.reshape(128, n_blocks, KQ, 2, 128).transpose(2, 0, 1, 3, 4)
        wv_in = np.ascontiguousarray(
            np.concatenate(
                [
                    x0_wave.reshape(KQ, 128, -1),
                    a_wave.reshape(KQ, 128, -1),
                    w_wave.reshape(KQ, 128, -1),
                ],
                axis=2,
            )
        )

        # lora tables: per block, B row | mask row
        lt_full = np.concatenate([b_pack, m_pack], axis=2)  # [128, nb, D+blk]
        lt_ins = [
            np.ascontiguousarray(lt_full[:, :sub].reshape(128, -1)),
            np.ascontiguousarray(lt_full[:, sub : 2 * sub].reshape(128, -1)),
            np.ascontiguousarray(lt_full[:, 2 * sub :].reshape(128, -1)),
        ]

        in_maps.append(
            {
                "wv": wv_in,
                "xr": xr_in,
                "lt0": lt_ins[0],
                "lt1": lt_ins[1],
                "lt2": lt_ins[2],
                "bias": bias_in,
            }
        )

    _last_in_maps = in_maps
    res = run_bass_kernel_spmd(nc, in_maps, core_ids=list(range(N_CORES)))

    out = np.empty((T, D), dtype=np.float32)
    for c in range(N_CORES):
        o_t = res.results[c]["outT"].reshape(D, TPC)  # [d, t] sorted tokens
        out[c * TPC + perms[c]] = o_t.T
    return out


# revision 15
# speedup vs baseline: 1.0233x; 1.0233x over previous
"""MoE-LoRA linear layer (T=16384, D=1024, E=64, R=8) on 8 Trainium2 cores.

Strategy: data-parallel over tokens (2048 tokens/core). Inside each core
everything is computed transposed (d on partitions, tokens on the free dim)
so every matmul consumes operands in their natural layout with no on-device
transposes:

  out_T[:, g] = sum_k W_k^T @ xT_k[:, g]      base GEMM, N=512 token groups
  out_T[:, b] += B_blk^T @ (mask_b * (A_blk^T @ xT[:, b]))   rank-8 LoRA

Routing is resolved on the host: each core's tokens are sorted by expert
label and cut into 256-token blocks; per block the (<=16) experts present
are packed into per-block A / B / mask tensors. The device program is thus
identical for all 8 cores (one SPMD NEFF) and all data-dependence lives in
input data. The LoRA matmul accumulates directly into the base GEMM's PSUM
tile (column sub-range), so composition costs no extra DVE work.

Schedule: the first token group's x/A/W stream as four combined k-pair
"waves" (one ~1.25MB DMA each) whose arrival rate matches PE consumption;
later groups' x and the B/mask tables arrive as single DMAs ordered by
first use. Throwaway warm-up matmuls bridge the fixed ~7.5us framework
preamble so the PE clock gate (HAM, 1.2 -> 2.4 GHz) releases early and
never re-throttles. Compute in bf16 (f32 PSUM): fp32 matmul on TRN2 runs
at 1/4 rate and would be hopelessly PE-bound.
"""

import numpy as np
import ml_dtypes

import concourse.bacc as bacc
import concourse.mybir as mybir
from concourse import tile
from concourse.bass_utils import run_bass_kernel_spmd

T, D, E, R = 16384, 1024, 64, 8
N_CORES = 8
TPC = T // N_CORES          # tokens per core
KD = D // 128               # 8 contraction chunks
KQ = KD // 2                # k-pair waves for the first group
GRP = 512                   # base-GEMM token group (one PSUM bank)
NG = TPC // GRP             # 4 groups
SCALING = 1.0 / R
SLOTS = 128 // R            # experts per lora block the packed layout holds

BF16 = ml_dtypes.bfloat16

_compiled = {}              # n_blocks -> Bacc program (reused across calls)
_last_in_maps = None


def _build_nc(n_blocks: int):
    blk = TPC // n_blocks   # lora block (256 default)
    sub = GRP // blk        # lora blocks per token group
    WV = 2 * GRP + n_blocks * 2 * 128 + 2 * D   # combined wave row: x | A | W
    LB = D + blk                                 # lora-table row per block: B | M
    bf = mybir.dt.bfloat16
    f32 = mybir.dt.float32

    nc = bacc.Bacc(
        "TRN2", target_bir_lowering=False, debug=False, num_devices=N_CORES
    )
    # host-packed SBUF layouts; every DMA source is contiguous per partition
    wv_d = nc.dram_tensor("wv", [KQ, 128, WV], bf, kind="ExternalInput")
    xr_d = nc.dram_tensor("xr", [NG - 1, 128, KD, GRP], bf, kind="ExternalInput")
    # lora tables in three pieces by first use: group0, group1, groups 2-3
    lt_shapes = [sub, sub, n_blocks - 2 * sub]
    lt_d = [
        nc.dram_tensor(f"lt{i}", [128, n * LB], bf, kind="ExternalInput")
        for i, n in enumerate(lt_shapes)
    ]
    bias_d = nc.dram_tensor("bias", [128, KD], f32, kind="ExternalInput")
    out_d = nc.dram_tensor("outT", [KD, 128, TPC], f32, kind="ExternalOutput")

    with tile.TileContext(nc) as tc:
        with (
            tc.tile_pool(name="consts", bufs=1) as cpool,
            tc.tile_pool(name="xa_ps", bufs=3, space="PSUM") as xa_ps,
            tc.tile_pool(name="out_ps", bufs=5, space="PSUM") as out_ps,
            tc.tile_pool(name="stage", bufs=4) as stage_pool,
        ):
            bias_t = cpool.tile([128, KD], f32, tag="bias", name="bias_t")
            wv_t = [
                cpool.tile([128, WV], bf, tag=f"wv{q}", name=f"wv_t{q}")
                for q in range(KQ)
            ]
            xr_t = [
                cpool.tile([128, KD * GRP], bf, tag=f"xr{g}", name=f"xr_t{g}")
                for g in range(1, NG)
            ]
            lt_t = [
                cpool.tile([128, n * LB], bf, tag=f"lt{i}", name=f"lt_t{i}")
                for i, n in enumerate(lt_shapes)
            ]
            warm_sb = cpool.tile([128, GRP], bf, tag="warm", name="warm_sb")

            A_OFF = 2 * GRP
            W_OFF = 2 * GRP + n_blocks * 2 * 128

            def a_sl(b, k):
                q, kk = divmod(k, 2)
                o = A_OFF + (b * 2 + kk) * 128
                return wv_t[q][:, o : o + 128]

            def w_sl(k, j):
                q, kk = divmod(k, 2)
                o = W_OFF + kk * D + j * 128
                return wv_t[q][:, o : o + 128]

            def x_sl(g, k, c0, c1):
                if g == 0:
                    q, kk = divmod(k, 2)
                    return wv_t[q][:, kk * GRP + c0 : kk * GRP + c1]
                return xr_t[g - 1][:, k * GRP + c0 : k * GRP + c1]

            def _lt(b):
                if b < sub:
                    return lt_t[0], b
                if b < 2 * sub:
                    return lt_t[1], b - sub
                return lt_t[2], b - 2 * sub

            def b_sl(b, j):
                t, lb = _lt(b)
                o = lb * LB + j * 128
                return t[:, o : o + 128]

            def m_sl(b):
                t, lb = _lt(b)
                o = lb * LB + D
                return t[:, o : o + blk]

            # issue order == arrival order (one sequencer queue)
            nc.sync.dma_start(bias_t[:], bias_d[:, :])
            for q in range(KQ):
                nc.sync.dma_start(wv_t[q][:], wv_d[q, :, :])
            nc.sync.dma_start(lt_t[0][:], lt_d[0][:, :])
            nc.sync.dma_start(xr_t[0][:], xr_d[0, :, :, :])
            nc.sync.dma_start(lt_t[1][:], lt_d[1][:, :])
            nc.sync.dma_start(xr_t[1][:], xr_d[1, :, :, :])
            nc.sync.dma_start(lt_t[2][:], lt_d[2][:, :])
            nc.sync.dma_start(xr_t[2][:], xr_d[2, :, :, :])

            # PE warm-up across the fixed framework preamble
            nc.vector.memset(warm_sb[:], 0.0)
            for _ in range(16):
                warm_ps = xa_ps.tile([128, GRP], f32, tag="xa", name="warm_ps")
                nc.tensor.matmul(
                    warm_ps[:],
                    lhsT=warm_sb[:, 0:128],
                    rhs=warm_sb[:],
                    start=True,
                    stop=True,
                    skip_group_check=True,
                )

            xa_m = [None] * n_blocks
            xa_p = [None] * n_blocks

            def emit_xa(b, ks, masked):
                # xa[slot, t] for lora block b over k-chunks ks
                g, h = divmod(b, sub)
                if ks[0] == 0:
                    xa_p[b] = xa_ps.tile([128, blk], f32, tag="xa", name=f"xa_p{b}")
                for k in ks:
                    nc.tensor.matmul(
                        xa_p[b][:],
                        lhsT=a_sl(b, k),
                        rhs=x_sl(g, k, h * blk, (h + 1) * blk),
                        start=(k == 0),
                        stop=(k == KD - 1),
                    )
                if masked:
                    xm = cpool.tile([128, blk], bf, tag=f"xam{b}", name=f"xm{b}")
                    nc.vector.tensor_mul(xm[:], xa_p[b][:], m_sl(b))
                    xa_m[b] = xm

            def emit_base(g, j, o_p, ks):
                for k in ks:
                    nc.tensor.matmul(
                        o_p[:],
                        lhsT=w_sl(k, j),
                        rhs=x_sl(g, k, 0, GRP),
                        start=(k == 0),
                        stop=False,
                        skip_group_check=True,
                    )

            def emit_lora_bias(g, j, o_p):
                for h in range(sub):
                    b = g * sub + h
                    nc.tensor.matmul(
                        o_p[:, h * blk : (h + 1) * blk],
                        lhsT=b_sl(b, j),
                        rhs=xa_m[b][:],
                        start=False,
                        stop=(h == sub - 1),
                        skip_group_check=True,
                    )
                st = stage_pool.tile([128, GRP], f32, tag="st", name=f"st{g}_{j}")
                nc.vector.tensor_scalar_add(st[:], o_p[:], bias_t[:, j : j + 1])
                nc.sync.dma_start(out_d[j, :, g * GRP : (g + 1) * GRP], st[:])

            # --- group 0: wave schedule matched to DMA arrivals ---
            o_p0 = {}
            for j in range(5):
                o_p0[j] = out_ps.tile([128, GRP], f32, tag="o", name=f"o_p0_{j}")
            for q in range(KQ):
                ks = (2 * q, 2 * q + 1)
                for j in range(5):
                    emit_base(0, j, o_p0[j], ks)
                for b in range(sub):
                    emit_xa(b, ks, masked=(q == KQ - 1))
            for j in range(5):
                emit_lora_bias(0, j, o_p0[j])
            for j in range(5, KD):
                o_p = out_ps.tile([128, GRP], f32, tag="o", name=f"o_p0_{j}")
                emit_base(0, j, o_p, range(KD))
                emit_lora_bias(0, j, o_p)
                if j == 5:
                    for h in range(sub):
                        emit_xa(sub + h, range(KD), masked=True)

            # --- groups 1..3: straight pipeline ---
            for g in range(1, NG):
                for j in range(KD):
                    o_p = out_ps.tile([128, GRP], f32, tag="o", name=f"o_p{g}_{j}")
                    emit_base(g, j, o_p, range(KD))
                    emit_lora_bias(g, j, o_p)
                    if j == 3 and g < NG - 1:
                        # next group's xa, placed where its x tile has arrived
                        for h in range(sub):
                            emit_xa((g + 1) * sub + h, range(KD), masked=True)

    nc.compile()
    return nc


def _pick_n_blocks(labels: np.ndarray) -> int:
    for n_blocks in (8, 16, 32, 64, 128, 256):
        blk = TPC // n_blocks
        ok = True
        for c in range(N_CORES):
            sl = np.sort(labels[c * TPC : (c + 1) * TPC])
            for b in range(n_blocks):
                if len(np.unique(sl[b * blk : (b + 1) * blk])) > SLOTS:
                    ok = False
                    break
            if not ok:
                break
        if ok:
            return n_blocks
    raise ValueError("could not find a block size with <=16 experts per block")


def kernel(x, labels, W, A, B, bias):
    global _last_in_maps
    x = np.asarray(x, dtype=np.float32)
    labels_i = np.asarray(labels).astype(np.int64)
    W = np.asarray(W, dtype=np.float32)
    A = np.asarray(A, dtype=np.float32)
    B = np.asarray(B, dtype=np.float32)
    bias = np.asarray(bias, dtype=np.float32)

    n_blocks = _pick_n_blocks(labels_i)
    blk = TPC // n_blocks
    sub = GRP // blk

    if n_blocks not in _compiled:
        _compiled[n_blocks] = _build_nc(n_blocks)
    nc = _compiled[n_blocks]

    # w_wave[q, p, kk, :] = W[128*(2q+kk)+p, :]
    w_wave = W.reshape(KQ, 2, 128, D).transpose(0, 2, 1, 3).astype(BF16)
    bias_in = np.ascontiguousarray(bias.reshape(KD, 128).T)  # [128, KD] f32
    B_scaled = (B * SCALING).astype(np.float32)

    in_maps = []
    perms = []
    for c in range(N_CORES):
        lc = labels_i[c * TPC : (c + 1) * TPC]
        perm = np.argsort(lc, kind="stable")
        perms.append(perm)
        ls = lc[perm]                          # sorted labels
        xs = x[c * TPC : (c + 1) * TPC][perm]  # [TPC, D] sorted tokens

        # xt_full[k, p, g, t] = xs[g*GRP + t, 128k + p]
        xt_full = xs.astype(BF16).T.reshape(KD, 128, NG, GRP)
        # x0 wave part [q, p, kk, t]
        x0_wave = xt_full[:, :, 0, :].reshape(KQ, 2, 128, GRP).transpose(0, 2, 1, 3)
        xr_in = np.ascontiguousarray(
            xt_full[:, :, 1:, :].transpose(2, 1, 0, 3)    # [NG-1, 128, KD, GRP]
        )

        # packed per-block expert tables
        a_pack = np.zeros((128, n_blocks, KD, 128), dtype=BF16)
        b_pack = np.zeros((128, n_blocks, D), dtype=BF16)
        m_pack = np.zeros((128, n_blocks, blk), dtype=BF16)
        for b in range(n_blocks):
            seg = ls[b * blk : (b + 1) * blk]
            experts = np.unique(seg)
            assert len(experts) <= SLOTS
            for i, e in enumerate(experts):
                # lhsT slot: a_pack[p, b, k, 8i+r] = A[e, 128k+p, r]
                a_pack[:, b, :, i * R : (i + 1) * R] = A[e].reshape(
                    KD, 128, R
                ).transpose(1, 0, 2)
                b_pack[i * R : (i + 1) * R, b, :] = B_scaled[e]
                m_pack[i * R : (i + 1) * R, b, :] = (seg == e)[None, :]

        # wave[q] = x0 | A | W  (concat along free dim)
        a_wave = a_pack.reshape(128, n_blocks, KQ, 2, 128).transpose(2, 0, 1, 3, 4)
        wv_in = np.ascontiguousarray(
            np.concatenate(
                [
                    x0_wave.reshape(KQ, 128, -1),
                    a_wave.reshape(KQ, 128, -1),
                    w_wave.reshape(KQ, 128, -1),
                ],
                axis=2,
            )
        )

        # lora tables: per block, B row | mask row
        lt_full = np.concatenate([b_pack, m_pack], axis=2)  # [128, nb, D+blk]
        lt_ins = [
            np.ascontiguousarray(lt_full[:, :sub].reshape(128, -1)),
            np.ascontiguousarray(lt_full[:, sub : 2 * sub].reshape(128, -1)),
            np.ascontiguousarray(lt_full[:, 2 * sub :].reshape(128, -1)),
        ]

        in_maps.append(
            {
                "wv": wv_in,
                "xr": xr_in,
                "lt0": lt_ins[0],
                "lt1": lt_ins[1],
                "lt2": lt_ins[2],
                "bias": bias_in,
            }
        )

    _last_in_maps = in_maps
    res = run_bass_kernel_spmd(nc, in_maps, core_ids=list(range(N_CORES)))

    out = np.empty((T, D), dtype=np.float32)
    for c in range(N_CORES):
        o_t = res.results[c]["outT"].reshape(D, TPC)  # [d, t] sorted tokens
        out[c * TPC + perms[c]] = o_t.T
    return out
